# revision 28
# baseline (speedup 1.0000x reference)
"""Bahdanau-attention Bass kernel for 8 TRN2 NeuronCores (data-parallel over batch).

Shapes (hardcoded): B=128, S=1024, EH2=1024, DH=512, A=512.
Returns (context [B, EH2] f32, attn_weights [B, S] f32) matching the reference.

Strategy per core (16 batch rows per core, no cross-core communication):
  - Host ships encoder_outputs once, in bf16, transposed to [e, s] chunks
    ("encT") — the only layout needed: PE contracts over partitions for proj,
    and the context reduction runs on DVE along the free (s) dim of encT.
  - projT[a, s] = sum_e W_enc[e, a] * encT[e, s]   (64 MMs/row, K-dense, bf16)
  - energyT = tanh(projT + dec_projT[:, row] + b_attnT) via ACT with fused
    per-partition bias (a on partitions).
  - scores[1, s] = sum_a v[a] * energyT[a, s]      (8 MMs/row, M=1)
  - expm = exp(scores) * maskrow   (no max-subtraction: |scores| <= ||v||_1)
  - expb = broadcast expm across partitions via two K=1 PE matmuls (ones ⊗ expm),
    evacuated PSUM->SBUF bf16 on ACT; recip broadcast the same way.
  - ctxT[e-chunk] = sum_s (encT * recip) * expb via DVE scalar_tensor_tensor
    with accum_out — one fused pass per e-chunk, no second enc read from HBM.
  - ctxT [128, 8] -> PE transpose (identity) -> [8, 128] -> DMA out.
"""

import os

import numpy as np
import ml_dtypes

B, S, E, DH, A = 128, 1024, 1024, 512, 512
NCORES = 8

LAST_EXEC_NS = None

_NC_CACHE = {}


def _build_nc(rows):
    import concourse.tile as tile
    from concourse import bacc, mybir

    f32 = mybir.dt.float32
    bf16 = mybir.dt.bfloat16
    Tanh = mybir.ActivationFunctionType.Tanh
    Exp = mybir.ActivationFunctionType.Exp
    Ident = mybir.ActivationFunctionType.Identity
    AX = mybir.AxisListType.X
    MULT = mybir.AluOpType.mult

    nc = bacc.Bacc(
        "TRN2", target_bir_lowering=False, debug=False, num_devices=NCORES
    )

    encT_d = nc.declare_dram_parameter("encT", [rows, 128, 8, S], bf16, isOutput=False)
    w_d = nc.declare_dram_parameter("w", [128, 8, A], bf16, isOutput=False)
    wd_d = nc.declare_dram_parameter("wd", [128, 4, A], bf16, isOutput=False)
    dhT_d = nc.declare_dram_parameter("dhT", [128, 4, rows], bf16, isOutput=False)
    bcols_d = nc.declare_dram_parameter("bcols", [128, 4], f32, isOutput=False)
    vcols_d = nc.declare_dram_parameter("vcols", [128, 4], bf16, isOutput=False)
    maskf_d = nc.declare_dram_parameter("maskf", [rows, S], bf16, isOutput=False)
    ident_d = nc.declare_dram_parameter("ident", [128, 128], f32, isOutput=False)
    out_d = nc.declare_dram_parameter("out", [rows, E + S], f32, isOutput=True)
    attn_bounce = nc.dram_tensor("attn_bounce", [rows, S], bf16)

    with tile.TileContext(nc) as tc:
        with (
            tc.tile_pool(name="singles", bufs=1) as singles,
            tc.tile_pool(name="encT_pool", bufs=4) as encT_pool,
            tc.tile_pool(name="energy_pool", bufs=2) as energy_pool,
            tc.tile_pool(name="small", bufs=3) as small,
            tc.tile_pool(name="outp", bufs=3) as outp,
            tc.tile_pool(name="mmps", bufs=4, space="PSUM") as mmps,
            tc.tile_pool(name="vecps", bufs=2, space="PSUM") as vecps,
        ):
            w_sb = singles.tile([128, 8, A], bf16)
            nc.sync.dma_start(out=w_sb[:], in_=w_d[:])
            wd_sb = singles.tile([128, 4, A], bf16)
            nc.sync.dma_start(out=wd_sb[:], in_=wd_d[:])
            dhT_sb = singles.tile([128, 4, rows], bf16)
            nc.sync.dma_start(out=dhT_sb[:], in_=dhT_d[:])
            bcols_sb = singles.tile([128, 4], f32)
            nc.sync.dma_start(out=bcols_sb[:], in_=bcols_d[:])
            vcols_sb = singles.tile([128, 4], bf16)
            nc.sync.dma_start(out=vcols_sb[:], in_=vcols_d[:])
            ident_sb = singles.tile([128, 128], f32)
            nc.sync.dma_start(out=ident_sb[:], in_=ident_d[:])
            one11 = singles.tile([1, 1], bf16)
            nc.vector.memset(one11, 1.0)

            # dec_projT[a, r] + b_attnT[a], laid out [128(a%), 4(a-chunk), rows]
            dp_ps = vecps.tile([128, 4 * rows], f32, tag="v")
            for m in range(4):
                for k in range(4):
                    nc.tensor.matmul(
                        dp_ps[:, m * rows : (m + 1) * rows],
                        lhsT=wd_sb[:, k, m * 128 : (m + 1) * 128],
                        rhs=dhT_sb[:, k, :],
                        start=(k == 0),
                        stop=(k == 3),
                    )
            dpT_sb = singles.tile([128, 4, rows], f32)
            for m in range(4):
                nc.scalar.activation(
                    out=dpT_sb[:, m, :],
                    in_=dp_ps[:, m * rows : (m + 1) * rows],
                    func=Ident,
                    bias=bcols_sb[:, m : m + 1],
                    scale=1.0,
                )

            import concourse.bass as bass

            def finalize_row(r, ct_dve, ct_act):
                # transpose ctx accumulators -> rows and write out; called two
                # rows late so the PE never stalls on the reductions.
                ps_d = vecps.tile([5, 128], mybir.dt.float32, tag="v")
                nc.tensor.transpose(ps_d[:], ct_dve[:], ident_sb[:])
                ps_a = vecps.tile([3, 128], mybir.dt.float32, tag="v")
                nc.tensor.transpose(ps_a[:], ct_act[:], ident_sb[:])
                outc_d = outp.tile([5, 1, 128], mybir.dt.float32, tag="outcd")
                nc.scalar.copy(out=outc_d[:, 0, :], in_=ps_d[:])
                outc_a = outp.tile([3, 1, 128], mybir.dt.float32, tag="outca")
                nc.scalar.copy(out=outc_a[:, 0, :], in_=ps_a[:])
                nc.scalar.dma_start(
                    out=out_d[r : r + 1, 0:640].rearrange("o (j f) -> j o f", j=5),
                    in_=outc_d[:],
                )
                nc.scalar.dma_start(
                    out=out_d[r : r + 1, 640:1024].rearrange("o (j f) -> j o f", j=3),
                    in_=outc_a[:],
                )

            pending = []
            for r in range(rows):
                encT_t = encT_pool.tile([128, 8, S], bf16)
                for c in range(8):
                    nc.sync.dma_start(out=encT_t[:, c, :], in_=encT_d[r, :, c, :])

                # projT -> tanh -> energyT (bf16), a on partitions.
                # One energyT tile per a-chunk so the scores MMs only wait on
                # the chunk they read (Tile deps are per-tile).
                energyT = []
                for m in range(4):
                    et = energy_pool.tile([128, S], bf16, name=f"energyT{m}", tag=f"e{m}")
                    energyT.append(et)
                    for n in range(2):
                        mm = mmps.tile([128, 512], mybir.dt.float32, tag="mm")
                        for k in range(8):
                            nc.tensor.matmul(
                                mm[:],
                                lhsT=w_sb[:, k, m * 128 : (m + 1) * 128],
                                rhs=encT_t[:, k, n * 512 : (n + 1) * 512],
                                start=(k == 0),
                                stop=(k == 7),
                            )
                        nc.scalar.activation(
                            out=et[:, n * 512 : (n + 1) * 512],
                            in_=mm[:],
                            func=Tanh,
                            bias=dpT_sb[:, m, r : r + 1],
                            scale=1.0,
                        )

                if len(pending) >= 2:
                    finalize_row(*pending.pop(0))

                # scores flat [1, S]; additive log-mask folded in as a
                # 5th accumulation so exp+sum can fuse on ACT
                maskrow = small.tile([1, S], bf16)
                nc.sync.dma_start(out=maskrow[:], in_=maskf_d[r : r + 1, :])
                scores_ps = vecps.tile([1, S], mybir.dt.float32, tag="v")
                for n in range(2):
                    for m in range(4):
                        nc.tensor.matmul(
                            scores_ps[0:1, n * 512 : (n + 1) * 512],
                            lhsT=vcols_sb[:, m : m + 1],
                            rhs=energyT[m][:, n * 512 : (n + 1) * 512],
                            start=(m == 0),
                            stop=False,
                        )
                    nc.tensor.matmul(
                        scores_ps[0:1, n * 512 : (n + 1) * 512],
                        lhsT=one11[:],
                        rhs=maskrow[0:1, n * 512 : (n + 1) * 512],
                        start=False,
                        stop=True,
                    )

                expm = small.tile([1, S], mybir.dt.float32)
                srtile = small.tile([1, 8], mybir.dt.float32)
                ssum = srtile[0:1, 0:1]
                recip = srtile[0:1, 1:2]
                nc.scalar.activation(
                    out=expm[:], in_=scores_ps[0:1, :], func=Exp, accum_out=ssum
                )
                nc.vector.reciprocal(out=recip, in_=ssum)

                # attn output (f32) and normalized bf16 copy for the broadcast
                outa = outp.tile([1, S], mybir.dt.float32, tag="outa")
                nc.vector.tensor_scalar_mul(out=outa[:], in0=expm[:], scalar1=recip)
                nc.scalar.dma_start(out=out_d[r : r + 1, E : E + S], in_=outa[:])
                attnbf = small.tile([1, S], bf16)
                nc.vector.tensor_scalar_mul(out=attnbf[:], in0=expm[:], scalar1=recip)

                # broadcast normalized attn across partitions: bounce through
                # DRAM, then stride-0 partition-broadcast DMA back to SBUF
                nc.scalar.dma_start(out=attn_bounce[r : r + 1, :], in_=attnbf[:])
                expb_sb = small.tile([128, S], bf16)
                brow = attn_bounce[r : r + 1, :]
                bcast_ap = bass.AP(
                    tensor=brow.tensor,
                    offset=brow.offset,
                    ap=[[0, 128], brow.ap[-1]],
                )
                nc.scalar.dma_start(out=expb_sb[:], in_=bcast_ap)

                # ctxT[e%128, e-chunk] = sum_s encT * attn  (fused DVE pass).
                # Alternate scratch AND accumulator tiles so consecutive
                # passes have no WAW dependency (Tile inserts a completion
                # self-wait chain otherwise, ~1us per op).
                # ctx reduction split: DVE muls c=0..5 (+reduces 0..4),
                # GpSimd muls c=6,7, ACT reduces 5..7 via accum_out.
                # ct_dve = chunks 0..4, ct_act = chunks 5..7 (contiguous so
                # the output DMA stays affine).
                ct_dve = small.tile([128, 5], mybir.dt.float32, name="ct_dve", tag="ctd")
                ct_act = small.tile([128, 3], mybir.dt.float32, name="ct_act", tag="ctc")
                scratches = [
                    small.tile([128, S], bf16, name=f"scr{i}", tag=f"scr{i}", bufs=1)
                    for i in range(4)
                ]
                scrg = [
                    small.tile([128, S], bf16, name=f"scrg{i}", tag=f"scrg{i}", bufs=1)
                    for i in range(2)
                ]

                def ctx_mul(c):
                    nc.vector.tensor_mul(
                        scratches[c % 4][:], encT_t[:, c, :], expb_sb[:]
                    )

                def ctx_red(c):
                    nc.vector.reduce_sum(
                        out=ct_dve[:, c : c + 1],
                        in_=scratches[c % 4][:],
                        axis=AX,
                    )

                # gpsimd handles the two muls the DVE doesn't
                nc.gpsimd.tensor_mul(scrg[0][:], encT_t[:, 6, :], expb_sb[:])
                nc.gpsimd.tensor_mul(scrg[1][:], encT_t[:, 7, :], expb_sb[:])

                # DVE: software-pipelined mul/reduce for chunks 0..5
                ctx_mul(0)
                for c in range(1, 6):
                    ctx_mul(c)
                    if c - 1 <= 4:
                        ctx_red(c - 1)
                # ACT reduces for chunks 5, 6, 7 (inputs from DVE/gpsimd muls)
                nc.scalar.activation(
                    out=scratches[1][:],
                    in_=scratches[1][:],
                    func=Ident,
                    accum_out=ct_act[:, 0:1],
                )
                ctx_red(4)
                nc.scalar.activation(
                    out=scrg[0][:],
                    in_=scrg[0][:],
                    func=Ident,
                    accum_out=ct_act[:, 1:2],
                )
                nc.scalar.activation(
                    out=scrg[1][:],
                    in_=scrg[1][:],
                    func=Ident,
                    accum_out=ct_act[:, 2:3],
                )

                pending.append((r, ct_dve, ct_act))

            while pending:
                finalize_row(*pending.pop(0))

    nc.compile()
    return nc


def _prep_inputs(encoder_outputs, decoder_hidden, src_mask, W_attn, b_attn, v):
    bf16 = ml_dtypes.bfloat16
    b = encoder_outputs.shape[0]
    rows = b // NCORES

    enc_bf = np.asarray(encoder_outputs, dtype=np.float32).astype(bf16)
    # encT_a[b, p, c, s] = enc[b, s, c*128+p]
    encT_a = np.ascontiguousarray(enc_bf.reshape(b, S, 8, 128).transpose(0, 3, 2, 1))

    W = np.asarray(W_attn, dtype=np.float32)
    w_a = np.ascontiguousarray(W[:E].astype(bf16).reshape(8, 128, A).transpose(1, 0, 2))
    wd_a = np.ascontiguousarray(
        W[E:].astype(bf16).reshape(4, 128, A).transpose(1, 0, 2)
    )
    bcols = np.ascontiguousarray(
        np.asarray(b_attn, dtype=np.float32).reshape(4, 128).T
    )
    vcols = np.ascontiguousarray(
        np.asarray(v, dtype=np.float32).astype(bf16).reshape(4, 128).T
    )
    ident = np.eye(128, dtype=np.float32)

    dh = np.asarray(decoder_hidden, dtype=np.float32)
    # additive log-mask: 0 where kept, -1e4 where masked (exp -> exactly 0)
    maskf = np.where(np.asarray(src_mask) != 0, 0.0, -1e4).astype(bf16)

    in_maps = []
    for i in range(NCORES):
        sl = slice(i * rows, (i + 1) * rows)
        dh_sh = dh[sl]  # [rows, DH]
        dhT_a = np.ascontiguousarray(
            dh_sh.T.astype(bf16).reshape(4, 128, rows).transpose(1, 0, 2)
        )
        in_maps.append(
            {
                "encT": encT_a[sl],
                "w": w_a,
                "wd": wd_a,
                "dhT": dhT_a,
                "bcols": bcols,
                "vcols": vcols,
                "maskf": np.ascontiguousarray(maskf[sl]),
                "ident": ident,
            }
        )
    return in_maps, rows


def kernel(encoder_outputs, decoder_hidden, src_mask, W_attn, b_attn, v):
    global LAST_EXEC_NS
    from concourse.bass_utils import run_bass_kernel_spmd

    in_maps, rows = _prep_inputs(
        encoder_outputs, decoder_hidden, src_mask, W_attn, b_attn, v
    )

    if rows not in _NC_CACHE:
        _NC_CACHE[rows] = _build_nc(rows)
    nc = _NC_CACHE[rows]

    trace = os.environ.get("KERNEL_TRACE", "0") == "1"
    res = run_bass_kernel_spmd(nc, in_maps, core_ids=list(range(NCORES)), trace=trace)
    LAST_EXEC_NS = res.exec_time_ns

    ctx = np.concatenate([r["out"][:, :E] for r in res.results], axis=0)
    attn = np.concatenate([r["out"][:, E:] for r in res.results], axis=0)
    return ctx.astype(np.float32), attn.astype(np.float32)


# revision 29
# speedup vs baseline: 1.0034x; 1.0034x over previous
"""Bahdanau-attention Bass kernel for 8 TRN2 NeuronCores (data-parallel over batch).

Shapes (hardcoded): B=128, S=1024, EH2=1024, DH=512, A=512.
Returns (context [B, EH2] f32, attn_weights [B, S] f32) matching the reference.

Strategy per core (16 batch rows per core, no cross-core communication):
  - Host ships encoder_outputs once, in bf16, transposed to [e, s] chunks
    ("encT") — the only layout needed: PE contracts over partitions for proj,
    and the context reduction runs on DVE along the free (s) dim of encT.
  - projT[a, s] = sum_e W_enc[e, a] * encT[e, s]   (64 MMs/row, K-dense, bf16)
  - energyT = tanh(projT + dec_projT[:, row] + b_attnT) via ACT with fused
    per-partition bias (a on partitions).
  - scores[1, s] = sum_a v[a] * energyT[a, s]      (8 MMs/row, M=1)
  - expm = exp(scores) * maskrow   (no max-subtraction: |scores| <= ||v||_1)
  - expb = broadcast expm across partitions via two K=1 PE matmuls (ones ⊗ expm),
    evacuated PSUM->SBUF bf16 on ACT; recip broadcast the same way.
  - ctxT[e-chunk] = sum_s (encT * recip) * expb via DVE scalar_tensor_tensor
    with accum_out — one fused pass per e-chunk, no second enc read from HBM.
  - ctxT [128, 8] -> PE transpose (identity) -> [8, 128] -> DMA out.
"""

import os

import numpy as np
import ml_dtypes

B, S, E, DH, A = 128, 1024, 1024, 512, 512
NCORES = 8

LAST_EXEC_NS = None

_NC_CACHE = {}


def _build_nc(rows):
    import concourse.tile as tile
    from concourse import bacc, mybir

    f32 = mybir.dt.float32
    bf16 = mybir.dt.bfloat16
    Tanh = mybir.ActivationFunctionType.Tanh
    Exp = mybir.ActivationFunctionType.Exp
    Ident = mybir.ActivationFunctionType.Identity
    AX = mybir.AxisListType.X
    MULT = mybir.AluOpType.mult

    nc = bacc.Bacc(
        "TRN2", target_bir_lowering=False, debug=False, num_devices=NCORES
    )

    encT_d = nc.declare_dram_parameter("encT", [rows, 128, 8, S], bf16, isOutput=False)
    w_d = nc.declare_dram_parameter("w", [128, 8, A], bf16, isOutput=False)
    wd_d = nc.declare_dram_parameter("wd", [128, 4, A], bf16, isOutput=False)
    dhT_d = nc.declare_dram_parameter("dhT", [128, 4, rows], bf16, isOutput=False)
    bcols_d = nc.declare_dram_parameter("bcols", [128, 4], f32, isOutput=False)
    vcols_d = nc.declare_dram_parameter("vcols", [128, 4], bf16, isOutput=False)
    maskf_d = nc.declare_dram_parameter("maskf", [rows, S], bf16, isOutput=False)
    ident_d = nc.declare_dram_parameter("ident", [128, 128], f32, isOutput=False)
    out_d = nc.declare_dram_parameter("out", [rows, E + S], f32, isOutput=True)
    attn_bounce = nc.dram_tensor("attn_bounce", [rows, S], bf16)

    with tile.TileContext(nc) as tc:
        with (
            tc.tile_pool(name="singles", bufs=1) as singles,
            tc.tile_pool(name="encT_pool", bufs=4) as encT_pool,
            tc.tile_pool(name="energy_pool", bufs=2) as energy_pool,
            tc.tile_pool(name="small", bufs=3) as small,
            tc.tile_pool(name="outp", bufs=3) as outp,
            tc.tile_pool(name="mmps", bufs=4, space="PSUM") as mmps,
            tc.tile_pool(name="vecps", bufs=2, space="PSUM") as vecps,
        ):
            w_sb = singles.tile([128, 8, A], bf16)
            nc.sync.dma_start(out=w_sb[:], in_=w_d[:])
            wd_sb = singles.tile([128, 4, A], bf16)
            nc.sync.dma_start(out=wd_sb[:], in_=wd_d[:])
            dhT_sb = singles.tile([128, 4, rows], bf16)
            nc.sync.dma_start(out=dhT_sb[:], in_=dhT_d[:])
            bcols_sb = singles.tile([128, 4], f32)
            nc.sync.dma_start(out=bcols_sb[:], in_=bcols_d[:])
            vcols_sb = singles.tile([128, 4], bf16)
            nc.sync.dma_start(out=vcols_sb[:], in_=vcols_d[:])
            ident_sb = singles.tile([128, 128], f32)
            nc.sync.dma_start(out=ident_sb[:], in_=ident_d[:])
            one11 = singles.tile([1, 1], bf16)
            nc.vector.memset(one11, 1.0)

            # dec_projT[a, r] + b_attnT[a], laid out [128(a%), 4(a-chunk), rows]
            dp_ps = vecps.tile([128, 4 * rows], f32, tag="v")
            for m in range(4):
                for k in range(4):
                    nc.tensor.matmul(
                        dp_ps[:, m * rows : (m + 1) * rows],
                        lhsT=wd_sb[:, k, m * 128 : (m + 1) * 128],
                        rhs=dhT_sb[:, k, :],
                        start=(k == 0),
                        stop=(k == 3),
                    )
            dpT_sb = singles.tile([128, 4, rows], f32)
            for m in range(4):
                nc.scalar.activation(
                    out=dpT_sb[:, m, :],
                    in_=dp_ps[:, m * rows : (m + 1) * rows],
                    func=Ident,
                    bias=bcols_sb[:, m : m + 1],
                    scale=1.0,
                )

            import concourse.bass as bass

            def finalize_row(r, ct_dve, ct_act):
                # transpose ctx accumulators -> rows and write out; called two
                # rows late so the PE never stalls on the reductions.
                ps_d = vecps.tile([6, 128], mybir.dt.float32, tag="v")
                nc.tensor.transpose(ps_d[:], ct_dve[:], ident_sb[:])
                ps_a = vecps.tile([2, 128], mybir.dt.float32, tag="v")
                nc.tensor.transpose(ps_a[:], ct_act[:], ident_sb[:])
                outc_d = outp.tile([6, 1, 128], mybir.dt.float32, tag="outcd")
                nc.scalar.copy(out=outc_d[:, 0, :], in_=ps_d[:])
                outc_a = outp.tile([2, 1, 128], mybir.dt.float32, tag="outca")
                nc.scalar.copy(out=outc_a[:, 0, :], in_=ps_a[:])
                nc.gpsimd.dma_start(
                    out=out_d[r : r + 1, 0:768].rearrange("o (j f) -> j o f", j=6),
                    in_=outc_d[:],
                )
                nc.gpsimd.dma_start(
                    out=out_d[r : r + 1, 768:1024].rearrange("o (j f) -> j o f", j=2),
                    in_=outc_a[:],
                )

            pending = []
            for r in range(rows):
                encT_t = encT_pool.tile([128, 8, S], bf16)
                for c in range(8):
                    nc.sync.dma_start(out=encT_t[:, c, :], in_=encT_d[r, :, c, :])

                # projT -> tanh -> energyT (bf16), a on partitions.
                # One energyT tile per a-chunk so the scores MMs only wait on
                # the chunk they read (Tile deps are per-tile).
                energyT = []
                for m in range(4):
                    et = energy_pool.tile([128, S], bf16, name=f"energyT{m}", tag=f"e{m}")
                    energyT.append(et)
                    for n in range(2):
                        mm = mmps.tile([128, 512], mybir.dt.float32, tag="mm")
                        for k in range(8):
                            nc.tensor.matmul(
                                mm[:],
                                lhsT=w_sb[:, k, m * 128 : (m + 1) * 128],
                                rhs=encT_t[:, k, n * 512 : (n + 1) * 512],
                                start=(k == 0),
                                stop=(k == 7),
                            )
                        nc.scalar.activation(
                            out=et[:, n * 512 : (n + 1) * 512],
                            in_=mm[:],
                            func=Tanh,
                            bias=dpT_sb[:, m, r : r + 1],
                            scale=1.0,
                        )

                if len(pending) >= 2:
                    finalize_row(*pending.pop(0))

                # scores flat [1, S]; additive log-mask folded in as a
                # 5th accumulation so exp+sum can fuse on ACT
                maskrow = small.tile([1, S], bf16)
                nc.sync.dma_start(out=maskrow[:], in_=maskf_d[r : r + 1, :])
                scores_ps = vecps.tile([1, S], mybir.dt.float32, tag="v")
                for n in range(2):
                    for m in range(4):
                        nc.tensor.matmul(
                            scores_ps[0:1, n * 512 : (n + 1) * 512],
                            lhsT=vcols_sb[:, m : m + 1],
                            rhs=energyT[m][:, n * 512 : (n + 1) * 512],
                            start=(m == 0),
                            stop=False,
                        )
                    nc.tensor.matmul(
                        scores_ps[0:1, n * 512 : (n + 1) * 512],
                        lhsT=one11[:],
                        rhs=maskrow[0:1, n * 512 : (n + 1) * 512],
                        start=False,
                        stop=True,
                    )

                expm = small.tile([1, S], mybir.dt.float32)
                srtile = small.tile([1, 8], mybir.dt.float32)
                ssum = srtile[0:1, 0:1]
                recip = srtile[0:1, 1:2]
                nc.scalar.activation(
                    out=expm[:], in_=scores_ps[0:1, :], func=Exp, accum_out=ssum
                )
                nc.vector.reciprocal(out=recip, in_=ssum)

                # attn output (f32) and normalized bf16 copy for the broadcast
                outa = outp.tile([1, S], mybir.dt.float32, tag="outa")
                nc.vector.tensor_scalar_mul(out=outa[:], in0=expm[:], scalar1=recip)
                nc.gpsimd.dma_start(out=out_d[r : r + 1, E : E + S], in_=outa[:])
                attnbf = small.tile([1, S], bf16)
                nc.vector.tensor_scalar_mul(out=attnbf[:], in0=expm[:], scalar1=recip)

                # broadcast normalized attn across partitions: bounce through
                # DRAM, then stride-0 partition-broadcast DMA back to SBUF
                nc.gpsimd.dma_start(out=attn_bounce[r : r + 1, :], in_=attnbf[:])
                expb_sb = small.tile([128, S], bf16)
                brow = attn_bounce[r : r + 1, :]
                bcast_ap = bass.AP(
                    tensor=brow.tensor,
                    offset=brow.offset,
                    ap=[[0, 128], brow.ap[-1]],
                )
                nc.gpsimd.dma_start(out=expb_sb[:], in_=bcast_ap)

                # ctxT[e%128, e-chunk] = sum_s encT * attn  (fused DVE pass).
                # Alternate scratch AND accumulator tiles so consecutive
                # passes have no WAW dependency (Tile inserts a completion
                # self-wait chain otherwise, ~1us per op).
                # ctx reduction: DVE muls all 8 chunks + reduces 0..5,
                # ACT reduces chunks 6,7 via accum_out. Contiguous chunk
                # split keeps the output DMA affine.
                ct_dve = small.tile([128, 6], mybir.dt.float32, name="ct_dve", tag="ctd")
                ct_act = small.tile([128, 2], mybir.dt.float32, name="ct_act", tag="ctc")
                scratches = [
                    small.tile([128, S], bf16, name=f"scr{i}", tag=f"scr{i}", bufs=1)
                    for i in range(4)
                ]

                def ctx_mul(c):
                    nc.vector.tensor_mul(
                        scratches[c % 4][:], encT_t[:, c, :], expb_sb[:]
                    )

                def ctx_red(c):
                    nc.vector.reduce_sum(
                        out=ct_dve[:, c : c + 1],
                        in_=scratches[c % 4][:],
                        axis=AX,
                    )

                # software-pipelined mul/reduce so same-engine completion
                # waits hide under the neighbouring op
                ctx_mul(0)
                for c in range(1, 8):
                    ctx_mul(c)
                    if c - 1 < 6:
                        ctx_red(c - 1)
                # ACT reduces for chunks 6, 7
                nc.scalar.activation(
                    out=scratches[2][:],
                    in_=scratches[2][:],
                    func=Ident,
                    accum_out=ct_act[:, 0:1],
                )
                nc.scalar.activation(
                    out=scratches[3][:],
                    in_=scratches[3][:],
                    func=Ident,
                    accum_out=ct_act[:, 1:2],
                )

                pending.append((r, ct_dve, ct_act))

            while pending:
                finalize_row(*pending.pop(0))

    nc.compile()
    return nc


def _prep_inputs(encoder_outputs, decoder_hidden, src_mask, W_attn, b_attn, v):
    bf16 = ml_dtypes.bfloat16
    b = encoder_outputs.shape[0]
    rows = b // NCORES

    enc_bf = np.asarray(encoder_outputs, dtype=np.float32).astype(bf16)
    # encT_a[b, p, c, s] = enc[b, s, c*128+p]
    encT_a = np.ascontiguousarray(enc_bf.reshape(b, S, 8, 128).transpose(0, 3, 2, 1))

    W = np.asarray(W_attn, dtype=np.float32)
    w_a = np.ascontiguousarray(W[:E].astype(bf16).reshape(8, 128, A).transpose(1, 0, 2))
    wd_a = np.ascontiguousarray(
        W[E:].astype(bf16).reshape(4, 128, A).transpose(1, 0, 2)
    )
    bcols = np.ascontiguousarray(
        np.asarray(b_attn, dtype=np.float32).reshape(4, 128).T
    )
    vcols = np.ascontiguousarray(
        np.asarray(v, dtype=np.float32).astype(bf16).reshape(4, 128).T
    )
    ident = np.eye(128, dtype=np.float32)

    dh = np.asarray(decoder_hidden, dtype=np.float32)
    # additive log-mask: 0 where kept, -1e4 where masked (exp -> exactly 0)
    maskf = np.where(np.asarray(src_mask) != 0, 0.0, -1e4).astype(bf16)

    in_maps = []
    for i in range(NCORES):
        sl = slice(i * rows, (i + 1) * rows)
        dh_sh = dh[sl]  # [rows, DH]
        dhT_a = np.ascontiguousarray(
            dh_sh.T.astype(bf16).reshape(4, 128, rows).transpose(1, 0, 2)
        )
        in_maps.append(
            {
                "encT": encT_a[sl],
                "w": w_a,
                "wd": wd_a,
                "dhT": dhT_a,
                "bcols": bcols,
                "vcols": vcols,
                "maskf": np.ascontiguousarray(maskf[sl]),
                "ident": ident,
            }
        )
    return in_maps, rows


def kernel(encoder_outputs, decoder_hidden, src_mask, W_attn, b_attn, v):
    global LAST_EXEC_NS
    from concourse.bass_utils import run_bass_kernel_spmd

    in_maps, rows = _prep_inputs(
        encoder_outputs, decoder_hidden, src_mask, W_attn, b_attn, v
    )

    if rows not in _NC_CACHE:
        _NC_CACHE[rows] = _build_nc(rows)
    nc = _NC_CACHE[rows]

    trace = os.environ.get("KERNEL_TRACE", "0") == "1"
    res = run_bass_kernel_spmd(nc, in_maps, core_ids=list(range(NCORES)), trace=trace)
    LAST_EXEC_NS = res.exec_time_ns

    ctx = np.concatenate([r["out"][:, :E] for r in res.results], axis=0)
    attn = np.concatenate([r["out"][:, E:] for r in res.results], axis=0)
    return ctx.astype(np.float32), attn.astype(np.float32)


# revision 31
# speedup vs baseline: 1.1293x; 1.1255x over previous
"""Bahdanau-attention Bass kernel for 8 TRN2 NeuronCores (data-parallel over batch).

Shapes (hardcoded): B=128, S=1024, EH2=1024, DH=512, A=512.
Returns (context [B, EH2] f32, attn_weights [B, S] f32) matching the reference.

Per core (16 batch rows, no cross-core communication), everything heavy on PE:
  - Host ships encoder_outputs twice in bf16: transposed [e, s] chunks (proj
    matmul contracts over partitions) and natural [s, e] chunks (context
    matmul contracts over s). Host prep is free w.r.t. HW exec time.
  - projT[a, s] = sum_e W_enc[e, a] * encT[e, s]  (64 MMs/row, bf16)
  - energyT = tanh(projT + dec_projT[:, row] + b_attnT), fused bias on ACT,
    one tile per a-chunk so scores only wait on the chunk they read.
  - scores[1, s] = sum_a v[a] * energyT[a, s] (8 MMs, M=1)
    (+ optional additive log-mask as an extra accumulation when the mask
    isn't all-ones)
  - exp on ACT with accum_out -> sum; reciprocal on DVE.
  - expT[s%128, s-chunk] via 8 K=1 matmuls (row -> partitions).
  - ctx[1, e] = sum_s expT-weighted enc (16 MMs, M=1, accumulate over s).
  - The expT + ctx matmuls for row r-1 are emitted inside row r's PE stream
    (after proj, around scores) so the in-order PE never waits on the
    softmax chain.
"""

import os

import numpy as np
import ml_dtypes

B, S, E, DH, A = 128, 1024, 1024, 512, 512
NCORES = 8

LAST_EXEC_NS = None

_NC_CACHE = {}


def _build_nc(rows, has_mask):
    import concourse.tile as tile
    from concourse import bacc, mybir

    f32 = mybir.dt.float32
    bf16 = mybir.dt.bfloat16
    Tanh = mybir.ActivationFunctionType.Tanh
    Exp = mybir.ActivationFunctionType.Exp
    Ident = mybir.ActivationFunctionType.Identity

    nc = bacc.Bacc(
        "TRN2", target_bir_lowering=False, debug=False, num_devices=NCORES
    )

    encT_d = nc.declare_dram_parameter("encT", [rows, 128, 8, S], bf16, isOutput=False)
    enc_d = nc.declare_dram_parameter("enc", [rows, 128, 8, E], bf16, isOutput=False)
    w_d = nc.declare_dram_parameter("w", [128, 8, A], bf16, isOutput=False)
    wd_d = nc.declare_dram_parameter("wd", [128, 4, A], bf16, isOutput=False)
    dhT_d = nc.declare_dram_parameter("dhT", [128, 4, rows], bf16, isOutput=False)
    bcols_d = nc.declare_dram_parameter("bcols", [128, 4], f32, isOutput=False)
    vcols_d = nc.declare_dram_parameter("vcols", [128, 4], bf16, isOutput=False)
    if has_mask:
        maskf_d = nc.declare_dram_parameter("maskf", [rows, S], bf16, isOutput=False)
    out_d = nc.declare_dram_parameter("out", [rows, E + S], f32, isOutput=True)

    with tile.TileContext(nc) as tc:
        with (
            tc.tile_pool(name="singles", bufs=1) as singles,
            tc.tile_pool(name="encT_pool", bufs=3) as encT_pool,
            tc.tile_pool(name="enc_pool", bufs=3) as enc_pool,
            tc.tile_pool(name="energy_pool", bufs=2) as energy_pool,
            tc.tile_pool(name="small", bufs=3) as small,
            tc.tile_pool(name="outp", bufs=3) as outp,
            tc.tile_pool(name="mmps", bufs=4, space="PSUM") as mmps,
            tc.tile_pool(name="vecps", bufs=2, space="PSUM") as vecps,
        ):
            w_sb = singles.tile([128, 8, A], bf16)
            nc.sync.dma_start(out=w_sb[:], in_=w_d[:])
            wd_sb = singles.tile([128, 4, A], bf16)
            nc.sync.dma_start(out=wd_sb[:], in_=wd_d[:])
            dhT_sb = singles.tile([128, 4, rows], bf16)
            nc.sync.dma_start(out=dhT_sb[:], in_=dhT_d[:])
            bcols_sb = singles.tile([128, 4], f32)
            nc.sync.dma_start(out=bcols_sb[:], in_=bcols_d[:])
            vcols_sb = singles.tile([128, 4], bf16)
            nc.sync.dma_start(out=vcols_sb[:], in_=vcols_d[:])
            if has_mask:
                one11 = singles.tile([1, 1], bf16)
                nc.vector.memset(one11, 1.0)

            # dec_projT[a, r] + b_attnT[a]: [128(a%), 4(a-chunk), rows]
            dp_ps = vecps.tile([128, 4 * rows], f32, tag="v")
            for m in range(4):
                for k in range(4):
                    nc.tensor.matmul(
                        dp_ps[:, m * rows : (m + 1) * rows],
                        lhsT=wd_sb[:, k, m * 128 : (m + 1) * 128],
                        rhs=dhT_sb[:, k, :],
                        start=(k == 0),
                        stop=(k == 3),
                    )
            dpT_sb = singles.tile([128, 4, rows], f32)
            for m in range(4):
                nc.scalar.activation(
                    out=dpT_sb[:, m, :],
                    in_=dp_ps[:, m * rows : (m + 1) * rows],
                    func=Ident,
                    bias=bcols_sb[:, m : m + 1],
                    scale=1.0,
                )

            def emit_expT(st):
                # transpose expm row -> partitions via K=1 matmuls (PE)
                st["expT_ps"] = vecps.tile(
                    [128, 8], mybir.dt.float32, tag="v", name="expT_ps"
                )
                for c in range(8):
                    nc.tensor.matmul(
                        st["expT_ps"][:, c : c + 1],
                        lhsT=st["expm"][0:1, c * 128 : (c + 1) * 128],
                        rhs=st["one_f"][:],
                        start=True,
                        stop=True,
                    )

            def emit_expT_copy(st):
                st["expT_sb"] = small.tile([128, 8], bf16, name="expT_sb")
                nc.vector.tensor_copy(out=st["expT_sb"][:], in_=st["expT_ps"][:])

            def emit_ctx(st):
                # ctx[1, e] accumulated over s-chunks (PE)
                r, enc_t = st["r"], st["enc_t"]
                ctx_ps = vecps.tile([1, E], mybir.dt.float32, tag="v")
                for eh in range(2):
                    for c in range(8):
                        nc.tensor.matmul(
                            ctx_ps[0:1, eh * 512 : (eh + 1) * 512],
                            lhsT=st["expT_sb"][:, c : c + 1],
                            rhs=enc_t[:, c, eh * 512 : (eh + 1) * 512],
                            start=(c == 0),
                            stop=(c == 7),
                        )
                # normalize on ACT (PSUM -> SBUF), then DMA on gpsimd ring
                outctx = outp.tile([1, E], mybir.dt.float32, tag="outctx")
                nc.scalar.activation(
                    out=outctx[:], in_=ctx_ps[0:1, :], func=Ident, scale=st["recip"]
                )
                nc.gpsimd.dma_start(out=out_d[r : r + 1, 0:E], in_=outctx[:])

            pending = None
            for r in range(rows):
                encT_t = encT_pool.tile([128, 8, S], bf16)
                for c in range(8):
                    nc.sync.dma_start(out=encT_t[:, c, :], in_=encT_d[r, :, c, :])
                enc_t = enc_pool.tile([128, 8, E], bf16)
                for c in range(8):
                    nc.sync.dma_start(out=enc_t[:, c, :], in_=enc_d[r, :, c, :])

                if has_mask:
                    maskrow = small.tile([1, S], bf16)
                    nc.sync.dma_start(out=maskrow[:], in_=maskf_d[r : r + 1, :])

                # projT -> tanh -> energyT (bf16), a on partitions
                energyT = []
                for m in range(4):
                    et = energy_pool.tile(
                        [128, S], bf16, name=f"energyT{m}", tag=f"e{m}"
                    )
                    energyT.append(et)
                    for n in range(2):
                        mm = mmps.tile([128, 512], mybir.dt.float32, tag="mm")
                        for k in range(8):
                            nc.tensor.matmul(
                                mm[:],
                                lhsT=w_sb[:, k, m * 128 : (m + 1) * 128],
                                rhs=encT_t[:, k, n * 512 : (n + 1) * 512],
                                start=(k == 0),
                                stop=(k == 7),
                            )
                        nc.scalar.activation(
                            out=et[:, n * 512 : (n + 1) * 512],
                            in_=mm[:],
                            func=Tanh,
                            bias=dpT_sb[:, m, r : r + 1],
                            scale=1.0,
                        )

                # deferred row r-1: expT matmuls (PE) before scores
                if pending is not None:
                    emit_expT(pending)
                    emit_expT_copy(pending)

                # scores flat [1, S]
                scores_ps = vecps.tile([1, S], mybir.dt.float32, tag="v")
                for n in range(2):
                    for m in range(4):
                        nc.tensor.matmul(
                            scores_ps[0:1, n * 512 : (n + 1) * 512],
                            lhsT=vcols_sb[:, m : m + 1],
                            rhs=energyT[m][:, n * 512 : (n + 1) * 512],
                            start=(m == 0),
                            stop=(m == 3) and not has_mask,
                        )
                    if has_mask:
                        nc.tensor.matmul(
                            scores_ps[0:1, n * 512 : (n + 1) * 512],
                            lhsT=one11[:],
                            rhs=maskrow[0:1, n * 512 : (n + 1) * 512],
                            start=False,
                            stop=True,
                        )

                # deferred row r-1: ctx matmuls (PE)
                if pending is not None:
                    emit_ctx(pending)
                    pending = None

                # softmax smalls
                expm = small.tile([1, S], mybir.dt.float32)
                srtile = small.tile([1, 8], mybir.dt.float32)
                ssum = srtile[0:1, 0:1]
                recip = srtile[0:1, 1:2]
                nc.scalar.activation(
                    out=expm[:], in_=scores_ps[0:1, :], func=Exp, accum_out=ssum
                )
                nc.vector.reciprocal(out=recip, in_=ssum)
                one_f = small.tile([1, 1], mybir.dt.float32, name="one_f")
                nc.vector.memset(one_f, 1.0)

                # attn output
                outa = outp.tile([1, S], mybir.dt.float32, tag="outa")
                nc.vector.tensor_scalar_mul(out=outa[:], in0=expm[:], scalar1=recip)
                nc.gpsimd.dma_start(out=out_d[r : r + 1, E : E + S], in_=outa[:])

                pending = {
                    "r": r,
                    "enc_t": enc_t,
                    "expm": expm,
                    "recip": recip,
                    "one_f": one_f,
                }

            if pending is not None:
                emit_expT(pending)
                emit_expT_copy(pending)
                emit_ctx(pending)

    nc.compile()
    return nc


def _prep_inputs(encoder_outputs, decoder_hidden, src_mask, W_attn, b_attn, v):
    bf16 = ml_dtypes.bfloat16
    b = encoder_outputs.shape[0]
    rows = b // NCORES

    enc_bf = np.asarray(encoder_outputs, dtype=np.float32).astype(bf16)
    # encT_a[b, p, c, s] = enc[b, s, c*128+p]
    encT_a = np.ascontiguousarray(enc_bf.reshape(b, S, 8, 128).transpose(0, 3, 2, 1))
    # enc_a[b, p, c, e] = enc[b, c*128+p, e]
    enc_a = np.ascontiguousarray(enc_bf.reshape(b, 8, 128, E).transpose(0, 2, 1, 3))

    W = np.asarray(W_attn, dtype=np.float32)
    w_a = np.ascontiguousarray(W[:E].astype(bf16).reshape(8, 128, A).transpose(1, 0, 2))
    wd_a = np.ascontiguousarray(
        W[E:].astype(bf16).reshape(4, 128, A).transpose(1, 0, 2)
    )
    bcols = np.ascontiguousarray(
        np.asarray(b_attn, dtype=np.float32).reshape(4, 128).T
    )
    vcols = np.ascontiguousarray(
        np.asarray(v, dtype=np.float32).astype(bf16).reshape(4, 128).T
    )

    dh = np.asarray(decoder_hidden, dtype=np.float32)
    mask_np = np.asarray(src_mask)
    has_mask = bool((mask_np == 0).any())
    # additive log-mask: 0 where kept, -1e4 where masked (exp -> exactly 0)
    maskf = np.where(mask_np != 0, 0.0, -1e4).astype(bf16)

    in_maps = []
    for i in range(NCORES):
        sl = slice(i * rows, (i + 1) * rows)
        dh_sh = dh[sl]  # [rows, DH]
        dhT_a = np.ascontiguousarray(
            dh_sh.T.astype(bf16).reshape(4, 128, rows).transpose(1, 0, 2)
        )
        m = {
            "encT": encT_a[sl],
            "enc": enc_a[sl],
            "w": w_a,
            "wd": wd_a,
            "dhT": dhT_a,
            "bcols": bcols,
            "vcols": vcols,
        }
        if has_mask:
            m["maskf"] = np.ascontiguousarray(maskf[sl])
        in_maps.append(m)
    return in_maps, rows, has_mask


def kernel(encoder_outputs, decoder_hidden, src_mask, W_attn, b_attn, v):
    global LAST_EXEC_NS
    from concourse.bass_utils import run_bass_kernel_spmd

    in_maps, rows, has_mask = _prep_inputs(
        encoder_outputs, decoder_hidden, src_mask, W_attn, b_attn, v
    )

    key = (rows, has_mask)
    if key not in _NC_CACHE:
        _NC_CACHE[key] = _build_nc(rows, has_mask)
    nc = _NC_CACHE[key]

    trace = os.environ.get("KERNEL_TRACE", "0") == "1"
    res = run_bass_kernel_spmd(nc, in_maps, core_ids=list(range(NCORES)), trace=trace)
    LAST_EXEC_NS = res.exec_time_ns

    ctx = np.concatenate([r["out"][:, :E] for r in res.results], axis=0)
    attn = np.concatenate([r["out"][:, E:] for r in res.results], axis=0)
    return ctx.astype(np.float32), attn.astype(np.float32)


# revision 33
# speedup vs baseline: 1.1745x; 1.0400x over previous
"""Bahdanau-attention Bass kernel for 8 TRN2 NeuronCores (data-parallel over batch).

Shapes (hardcoded): B=128, S=1024, EH2=1024, DH=512, A=512.
Returns (context [B, EH2] f32, attn_weights [B, S] f32) matching the reference.

Per core (16 batch rows, no cross-core communication), everything heavy on PE:
  - Host ships encoder_outputs twice in bf16: transposed [e, s] chunks (proj
    matmul contracts over partitions) and natural [s, e] chunks (context
    matmul contracts over s). Host prep is free w.r.t. HW exec time.
  - projT[a, s] = sum_e W_enc[e, a] * encT[e, s]  (64 MMs/row, bf16)
  - energyT = tanh(projT + dec_projT[:, row] + b_attnT), fused bias on ACT,
    one tile per a-chunk so scores only wait on the chunk they read.
  - scores[1, s] = sum_a v[a] * energyT[a, s] (8 MMs, M=1)
    (+ optional additive log-mask as an extra accumulation when the mask
    isn't all-ones)
  - exp on ACT with accum_out -> sum; reciprocal on DVE.
  - expT[s%128, s-chunk] via 8 K=1 matmuls (row -> partitions).
  - ctx[1, e] = sum_s expT-weighted enc (16 MMs, M=1, accumulate over s).
  - The expT + ctx matmuls for row r-1 are emitted inside row r's PE stream
    (after proj, around scores) so the in-order PE never waits on the
    softmax chain.
"""

import os

import numpy as np
import ml_dtypes

B, S, E, DH, A = 128, 1024, 1024, 512, 512
NCORES = 8

LAST_EXEC_NS = None

_NC_CACHE = {}


def _build_nc(rows, has_mask):
    import concourse.tile as tile
    from concourse import bacc, mybir

    f32 = mybir.dt.float32
    bf16 = mybir.dt.bfloat16
    Tanh = mybir.ActivationFunctionType.Tanh
    Exp = mybir.ActivationFunctionType.Exp
    Ident = mybir.ActivationFunctionType.Identity

    nc = bacc.Bacc(
        "TRN2", target_bir_lowering=False, debug=False, num_devices=NCORES
    )

    encT_d = nc.declare_dram_parameter("encT", [rows, 128, 8, S], bf16, isOutput=False)
    w_d = nc.declare_dram_parameter("w", [128, 8, A], bf16, isOutput=False)
    wd_d = nc.declare_dram_parameter("wd", [128, 4, A], bf16, isOutput=False)
    dhT_d = nc.declare_dram_parameter("dhT", [128, 4, rows], bf16, isOutput=False)
    bcols_d = nc.declare_dram_parameter("bcols", [128, 4], f32, isOutput=False)
    vcols_d = nc.declare_dram_parameter("vcols", [128, 4], bf16, isOutput=False)
    ident_d = nc.declare_dram_parameter("ident", [128, 128], f32, isOutput=False)
    if has_mask:
        maskf_d = nc.declare_dram_parameter("maskf", [rows, S], bf16, isOutput=False)
    out_d = nc.declare_dram_parameter("out", [rows, E + S], f32, isOutput=True)
    attn_bounce = nc.dram_tensor("attn_bounce", [rows, S], bf16)

    with tile.TileContext(nc) as tc:
        with (
            tc.tile_pool(name="singles", bufs=1) as singles,
            tc.tile_pool(name="encT_pool", bufs=6) as encT_pool,
            tc.tile_pool(name="energy_pool", bufs=2) as energy_pool,
            tc.tile_pool(name="small", bufs=3) as small,
            tc.tile_pool(name="outp", bufs=3) as outp,
            tc.tile_pool(name="mmps", bufs=4, space="PSUM") as mmps,
            tc.tile_pool(name="vecps", bufs=2, space="PSUM") as vecps,
        ):
            w_sb = singles.tile([128, 8, A], bf16)
            nc.sync.dma_start(out=w_sb[:], in_=w_d[:])
            wd_sb = singles.tile([128, 4, A], bf16)
            nc.sync.dma_start(out=wd_sb[:], in_=wd_d[:])
            dhT_sb = singles.tile([128, 4, rows], bf16)
            nc.sync.dma_start(out=dhT_sb[:], in_=dhT_d[:])
            bcols_sb = singles.tile([128, 4], f32)
            nc.sync.dma_start(out=bcols_sb[:], in_=bcols_d[:])
            vcols_sb = singles.tile([128, 4], bf16)
            nc.sync.dma_start(out=vcols_sb[:], in_=vcols_d[:])
            ident_sb = singles.tile([128, 128], f32)
            nc.sync.dma_start(out=ident_sb[:], in_=ident_d[:])
            if has_mask:
                one11 = singles.tile([1, 1], bf16)
                nc.vector.memset(one11, 1.0)

            # dec_projT[a, r] + b_attnT[a]: [128(a%), 4(a-chunk), rows]
            dp_ps = vecps.tile([128, 4 * rows], f32, tag="v")
            for m in range(4):
                for k in range(4):
                    nc.tensor.matmul(
                        dp_ps[:, m * rows : (m + 1) * rows],
                        lhsT=wd_sb[:, k, m * 128 : (m + 1) * 128],
                        rhs=dhT_sb[:, k, :],
                        start=(k == 0),
                        stop=(k == 3),
                    )
            dpT_sb = singles.tile([128, 4, rows], f32)
            for m in range(4):
                nc.scalar.activation(
                    out=dpT_sb[:, m, :],
                    in_=dp_ps[:, m * rows : (m + 1) * rows],
                    func=Ident,
                    bias=bcols_sb[:, m : m + 1],
                    scale=1.0,
                )

            import concourse.bass as bass

            def emit_bcast(st):
                # attnbf -> DRAM bounce -> stride-0 partition broadcast
                r = st["r"]
                nc.gpsimd.dma_start(
                    out=attn_bounce[r : r + 1, :], in_=st["attnbf"][:]
                )
                expb_sb = small.tile([128, S], bf16, name="expb_sb", bufs=3)
                st["expb_sb"] = expb_sb
                brow = attn_bounce[r : r + 1, :]
                bcast_ap = bass.AP(
                    tensor=brow.tensor,
                    offset=brow.offset,
                    ap=[[0, 128], brow.ap[-1]],
                )
                nc.gpsimd.dma_start(out=expb_sb[:], in_=bcast_ap)

            def emit_ctx_dve(st):
                # ctxT[e%128, chunk] = sum_s encT * attn on DVE (muls +
                # reduces 0..5) and ACT (reduces 6, 7), software-pipelined
                encT_t, expb_sb = st["encT_t"], st["expb_sb"]
                ct_dve = small.tile([128, 6], mybir.dt.float32, name="ct_dve", tag="ctd")
                ct_act = small.tile([128, 2], mybir.dt.float32, name="ct_act", tag="ctc")
                st["ct_dve"], st["ct_act"] = ct_dve, ct_act
                scr = [
                    small.tile([128, S], bf16, name=f"scr{i}", tag=f"scr{i}", bufs=1)
                    for i in range(4)
                ]

                def cmul(c):
                    nc.vector.tensor_mul(scr[c % 4][:], encT_t[:, c, :], expb_sb[:])

                def cred(c):
                    nc.vector.reduce_sum(
                        out=ct_dve[:, c : c + 1], in_=scr[c % 4][:], axis=mybir.AxisListType.X
                    )

                cmul(0)
                cmul(1)
                for c in range(2, 8):
                    cmul(c)
                    if c - 2 < 6:
                        cred(c - 2)
                for c in (6, 7):
                    nc.scalar.activation(
                        out=scr[c % 4][:],
                        in_=scr[c % 4][:],
                        func=Ident,
                        accum_out=ct_act[:, c - 6 : c - 5],
                    )

            def emit_ctx_out(st):
                # transposes on PE (deferred another row), copies + DMA out
                r = st["r"]
                ps_d = vecps.tile([6, 128], mybir.dt.float32, tag="v", name="ps_d")
                nc.tensor.transpose(ps_d[:], st["ct_dve"][:], ident_sb[:])
                ps_a = vecps.tile([2, 128], mybir.dt.float32, tag="v", name="ps_a")
                nc.tensor.transpose(ps_a[:], st["ct_act"][:], ident_sb[:])
                outc_d = outp.tile([6, 1, 128], mybir.dt.float32, tag="outcd")
                nc.scalar.copy(out=outc_d[:, 0, :], in_=ps_d[:])
                outc_a = outp.tile([2, 1, 128], mybir.dt.float32, tag="outca")
                nc.scalar.copy(out=outc_a[:, 0, :], in_=ps_a[:])
                nc.gpsimd.dma_start(
                    out=out_d[r : r + 1, 0:768].rearrange("o (j f) -> j o f", j=6),
                    in_=outc_d[:],
                )
                nc.gpsimd.dma_start(
                    out=out_d[r : r + 1, 768:1024].rearrange("o (j f) -> j o f", j=2),
                    in_=outc_a[:],
                )

            pend_out = []
            for r in range(rows):
                encT_t = encT_pool.tile([128, 8, S], bf16)
                for c in range(8):
                    nc.sync.dma_start(out=encT_t[:, c, :], in_=encT_d[r, :, c, :])
                if has_mask:
                    maskrow = small.tile([1, S], bf16)
                    nc.sync.dma_start(out=maskrow[:], in_=maskf_d[r : r + 1, :])

                # projT -> tanh -> energyT (bf16), a on partitions
                energyT = []
                for m in range(4):
                    et = energy_pool.tile(
                        [128, S], bf16, name=f"energyT{m}", tag=f"e{m}"
                    )
                    energyT.append(et)
                    for n in range(2):
                        mm = mmps.tile([128, 512], mybir.dt.float32, tag="mm")
                        for k in range(8):
                            nc.tensor.matmul(
                                mm[:],
                                lhsT=w_sb[:, k, m * 128 : (m + 1) * 128],
                                rhs=encT_t[:, k, n * 512 : (n + 1) * 512],
                                start=(k == 0),
                                stop=(k == 7),
                            )
                        nc.scalar.activation(
                            out=et[:, n * 512 : (n + 1) * 512],
                            in_=mm[:],
                            func=Tanh,
                            bias=dpT_sb[:, m, r : r + 1],
                            scale=1.0,
                        )

                # deferred PE transposes for row r-2 (DVE ctx is slow)
                if len(pend_out) >= 2:
                    emit_ctx_out(pend_out.pop(0))

                # scores flat [1, S]
                scores_ps = vecps.tile([1, S], mybir.dt.float32, tag="v")
                for n in range(2):
                    for m in range(4):
                        nc.tensor.matmul(
                            scores_ps[0:1, n * 512 : (n + 1) * 512],
                            lhsT=vcols_sb[:, m : m + 1],
                            rhs=energyT[m][:, n * 512 : (n + 1) * 512],
                            start=(m == 0),
                            stop=(m == 3) and not has_mask,
                        )
                    if has_mask:
                        nc.tensor.matmul(
                            scores_ps[0:1, n * 512 : (n + 1) * 512],
                            lhsT=one11[:],
                            rhs=maskrow[0:1, n * 512 : (n + 1) * 512],
                            start=False,
                            stop=True,
                        )

                # softmax smalls
                expm = small.tile([1, S], mybir.dt.float32)
                srtile = small.tile([1, 8], mybir.dt.float32)
                ssum = srtile[0:1, 0:1]
                recip = srtile[0:1, 1:2]
                nc.scalar.activation(
                    out=expm[:], in_=scores_ps[0:1, :], func=Exp, accum_out=ssum
                )
                nc.vector.reciprocal(out=recip, in_=ssum)

                # attn output (f32) + normalized bf16 copy for the broadcast
                outa = outp.tile([1, S], mybir.dt.float32, tag="outa")
                nc.vector.tensor_scalar_mul(out=outa[:], in0=expm[:], scalar1=recip)
                nc.gpsimd.dma_start(out=out_d[r : r + 1, E : E + S], in_=outa[:])
                attnbf = small.tile([1, S], bf16, name="attnbf")
                nc.vector.tensor_scalar_mul(out=attnbf[:], in0=expm[:], scalar1=recip)

                st = {"r": r, "encT_t": encT_t, "attnbf": attnbf}
                emit_bcast(st)
                emit_ctx_dve(st)
                pend_out.append(st)

            while pend_out:
                emit_ctx_out(pend_out.pop(0))

    nc.compile()
    return nc


def _prep_inputs(encoder_outputs, decoder_hidden, src_mask, W_attn, b_attn, v):
    bf16 = ml_dtypes.bfloat16
    b = encoder_outputs.shape[0]
    rows = b // NCORES

    enc_bf = np.asarray(encoder_outputs, dtype=np.float32).astype(bf16)
    # encT_a[b, p, c, s] = enc[b, s, c*128+p]
    encT_a = np.ascontiguousarray(enc_bf.reshape(b, S, 8, 128).transpose(0, 3, 2, 1))

    W = np.asarray(W_attn, dtype=np.float32)
    w_a = np.ascontiguousarray(W[:E].astype(bf16).reshape(8, 128, A).transpose(1, 0, 2))
    wd_a = np.ascontiguousarray(
        W[E:].astype(bf16).reshape(4, 128, A).transpose(1, 0, 2)
    )
    bcols = np.ascontiguousarray(
        np.asarray(b_attn, dtype=np.float32).reshape(4, 128).T
    )
    vcols = np.ascontiguousarray(
        np.asarray(v, dtype=np.float32).astype(bf16).reshape(4, 128).T
    )
    ident = np.eye(128, dtype=np.float32)

    dh = np.asarray(decoder_hidden, dtype=np.float32)
    mask_np = np.asarray(src_mask)
    has_mask = bool((mask_np == 0).any())
    # additive log-mask: 0 where kept, -1e4 where masked (exp -> exactly 0)
    maskf = np.where(mask_np != 0, 0.0, -1e4).astype(bf16)

    in_maps = []
    for i in range(NCORES):
        sl = slice(i * rows, (i + 1) * rows)
        dh_sh = dh[sl]  # [rows, DH]
        dhT_a = np.ascontiguousarray(
            dh_sh.T.astype(bf16).reshape(4, 128, rows).transpose(1, 0, 2)
        )
        m = {
            "encT": encT_a[sl],
            "w": w_a,
            "wd": wd_a,
            "dhT": dhT_a,
            "bcols": bcols,
            "vcols": vcols,
            "ident": ident,
        }
        if has_mask:
            m["maskf"] = np.ascontiguousarray(maskf[sl])
        in_maps.append(m)
    return in_maps, rows, has_mask


def kernel(encoder_outputs, decoder_hidden, src_mask, W_attn, b_attn, v):
    global LAST_EXEC_NS
    from concourse.bass_utils import run_bass_kernel_spmd

    in_maps, rows, has_mask = _prep_inputs(
        encoder_outputs, decoder_hidden, src_mask, W_attn, b_attn, v
    )

    key = (rows, has_mask)
    if key not in _NC_CACHE:
        _NC_CACHE[key] = _build_nc(rows, has_mask)
    nc = _NC_CACHE[key]

    trace = os.environ.get("KERNEL_TRACE", "0") == "1"
    res = run_bass_kernel_spmd(nc, in_maps, core_ids=list(range(NCORES)), trace=trace)
    LAST_EXEC_NS = res.exec_time_ns

    ctx = np.concatenate([r["out"][:, :E] for r in res.results], axis=0)
    attn = np.concatenate([r["out"][:, E:] for r in res.results], axis=0)
    return ctx.astype(np.float32), attn.astype(np.float32)
